# revision 20
# baseline (speedup 1.0000x reference)
"""GQA prefill attention (B=1, T=2048, DIM=4096, 32 q-heads / 8 kv-heads),
tensor-parallel over 8 NeuronCores.

Sharding: core c owns kv head c and its 4 query heads: wq rows
[512c, 512c+512), wk/wv rows [128c, 128c+128), wo cols [512c, 512c+512).
Each core computes a partial y = attn_c @ wo_c.T in [T, DIM]; the host sums
the 8 partials (the "all-reduce after wo").

All matmuls run in float32r (TF32-like; measured ~10% faster per row than
bf16 on this part). Everything on chip is kept transposed ([feature, t]
layouts) so that the PE contraction dim (partitions) always lines up without
any on-chip data transposes except V (16 PE-transposes of 128x128).

Per-core pipeline:
  phase 1: Q^T/K^T/V^T projections from x^T (host-pretransposed inputs);
           wo prefetched into SBUF during chunks 1-3
  phase 2: causal flash attention per head, s-on-partitions layout:
           S^T tiles -> exp (max-free softmax, scores are bounded) ->
           rowsum via ones-stationary matmul -> O^T accumulation ->
           reciprocal_approx_fast -> partition-broadcast via K=1 bf16
           matmul (no DRAM bounce) -> normalize into ao
  phase 3: y_partial = AO^T.T @ wo^T in [t, f] tiles, hb-outer psum
           accumulation over 8 banks, bf16 partials DMAd to DRAM
"""

import sys

sys.path.insert(0, "/opt/trn_rl_repo")

import ml_dtypes
import numpy as np

import concourse.bass as bass
import concourse.tile as tile
from concourse import bacc, mybir
from concourse.bass_utils import run_bass_kernel_spmd
from concourse.masks import make_identity

T = 2048
DIM = 4096
HD = 128
NCORE = 8
NH = 4  # q heads per core
TCH = 512
NTCH = T // TCH  # 4 t-chunks
NST = T // 128  # 16 s-tiles
NDT = DIM // 128  # 32 d-tiles
F32 = mybir.dt.float32
F32R = mybir.dt.float32r
BF16 = mybir.dt.bfloat16
SCALE = 1.0 / float(np.sqrt(HD))
NEG = -1e30

# test.py can flip these before calling kernel() to get profiling info
TRACE = False
LAST = {}

_CACHE = {}


def _build():
    nc = bacc.Bacc("TRN2", target_bir_lowering=False, debug=False, num_devices=NCORE)
    xT = nc.dram_tensor("xT", [DIM, T], F32, kind="ExternalInput").ap()
    wqT = nc.dram_tensor("wqT", [DIM, NH * HD], F32, kind="ExternalInput").ap()
    wkT = nc.dram_tensor("wkT", [DIM, HD], F32, kind="ExternalInput").ap()
    wvT = nc.dram_tensor("wvT", [DIM, HD], F32, kind="ExternalInput").ap()
    woT = nc.dram_tensor("woT", [NH * HD, DIM], F32, kind="ExternalInput").ap()
    ones_in = nc.dram_tensor("ones", [128, 1], F32, kind="ExternalInput").ap()
    onescol_in = nc.dram_tensor("onescol", [1, 128], BF16, kind="ExternalInput").ap()
    y = nc.dram_tensor("y", [T, DIM], BF16, kind="ExternalOutput").ap()

    with tile.TileContext(nc) as tc:
        with tc.tile_pool(name="persist", bufs=1) as persist:
            qt_sb = [persist.tile([128, T], F32R, tag=f"qt{h}", name=f"qt{h}") for h in range(NH)]
            kt_sb = persist.tile([128, T], F32R, tag="kt")
            vt_sb = persist.tile([128, T], F32, tag="vt")
            ao_sb = [persist.tile([128, T], F32R, tag=f"ao{h}", name=f"ao{h}") for h in range(NH)]
            ones_sb = persist.tile([128, 1], F32R, tag="ones")
            onescol = persist.tile([1, 128], BF16, tag="onescol")
            ident = persist.tile([128, 128], F32, tag="ident")
            v_sb = persist.tile([128, NST, HD], F32R, tag="v")
            nc.sync.dma_start(out=ones_sb, in_=ones_in.bitcast(F32R))
            nc.sync.dma_start(out=onescol, in_=onescol_in)
            make_identity(nc, ident)
            wor = woT.rearrange("(hb p) f -> p hb f", p=128)

            # ---------------- phase 1: Q/K/V projections ----------------
            with (
                tc.tile_pool(name="w1", bufs=1) as w1,
                tc.tile_pool(name="xs", bufs=6) as xs,
                tc.tile_pool(name="psp", bufs=1, space="PSUM") as psp,
                tc.tile_pool(name="ptr1", bufs=1, space="PSUM") as ptr1,
            ):
                wq_sb = w1.tile([128, NDT, NH * HD], F32R, tag="wq")
                wk_sb = w1.tile([128, NDT, HD], F32R, tag="wk")
                wv_sb = w1.tile([128, NDT, HD], F32R, tag="wv")
                wqr = wqT.rearrange("(db p) f -> p db f", p=128)
                wkr = wkT.rearrange("(db p) f -> p db f", p=128)
                wvr = wvT.rearrange("(db p) f -> p db f", p=128)
                for ch in range(NTCH):
                    cs = slice(ch * TCH, (ch + 1) * TCH)
                    qps = [
                        psp.tile([128, TCH], F32, tag=f"projq{fq}", name=f"projq{fq}")
                        for fq in range(NH)
                    ]
                    kps = psp.tile([128, TCH], F32, tag="projk")
                    vps = psp.tile([128, TCH], F32, tag="projv")
                    for d in range(NDT):
                        if ch == 0 and (d in (0, 1) or (d >= 4 and d % 4 == 0)):
                            # first two tiny groups so the PE starts almost
                            # immediately; 1MB groups once streaming
                            g = slice(d, d + (1 if d == 0 else 3 if d == 1 else 4))
                            nc.scalar.dma_start(
                                out=wq_sb[:, g, :], in_=wqr[:, g, :].bitcast(F32R)
                            )
                            nc.scalar.dma_start(
                                out=wk_sb[:, g, :], in_=wkr[:, g, :].bitcast(F32R)
                            )
                            nc.scalar.dma_start(
                                out=wv_sb[:, g, :], in_=wvr[:, g, :].bitcast(F32R)
                            )
                        xt = xs.tile([128, TCH], F32R, tag="xt")
                        nc.sync.dma_start(
                            out=xt,
                            in_=xT[d * 128 : (d + 1) * 128, cs].bitcast(F32R),
                        )
                        st = d == 0
                        sp = d == NDT - 1
                        for fq in range(NH):
                            nc.tensor.matmul(
                                qps[fq][:],
                                wq_sb[:, d, fq * HD : (fq + 1) * HD],
                                xt[:],
                                start=st,
                                stop=sp,
                            )
                        nc.tensor.matmul(
                            kps[:], wk_sb[:, d, :], xt[:], start=st, stop=sp
                        )
                        nc.tensor.matmul(
                            vps[:], wv_sb[:, d, :], xt[:], start=st, stop=sp
                        )
                    for fq in range(NH):
                        nc.vector.tensor_copy(qt_sb[fq][:, cs], qps[fq][:])
                    nc.vector.tensor_copy(kt_sb[:, cs], kps[:])
                    nc.vector.tensor_copy(vt_sb[:, cs], vps[:])
                    for ii in range(4 * ch, 4 * ch + 4):
                        ptr = ptr1.tile([128, HD], F32, tag="tr")
                        nc.tensor.transpose(
                            ptr[:], vt_sb[:, ii * 128 : (ii + 1) * 128], ident[:]
                        )
                        nc.vector.tensor_copy(v_sb[:, ii, :], ptr[:])

            # ---------------- phase 2 + 3 weights ----------------
            with tc.tile_pool(name="w2", bufs=1) as w2:
                wo_sb = w2.tile([128, NH, DIM], F32R, tag="wo")
                for hb in range(NH):
                    nc.scalar.dma_start(
                        out=wo_sb[:, hb, :], in_=wor[:, hb, :].bitcast(F32R)
                    )
                _phases23(nc, tc, qt_sb, kt_sb, v_sb, ao_sb, ones_sb, onescol,
                          wo_sb, y)

    nc.compile()
    return nc


def _phases23(nc, tc, qt_sb, kt_sb, v_sb, ao_sb, ones_sb, onescol, wo_sb, y):
    if True:
            # ---------------- phase 2: attention ----------------
            with (
                tc.tile_pool(name="phatp", bufs=6) as phatp,
                tc.tile_pool(name="recipp", bufs=2) as recipp,
                tc.tile_pool(name="recipbp", bufs=2) as recipbp,
                tc.tile_pool(name="rbcp", bufs=2) as rbcp,
                tc.tile_pool(name="pst", bufs=3, space="PSUM") as pst,
                tc.tile_pool(name="psl", bufs=1, space="PSUM") as psl,
                tc.tile_pool(name="psot", bufs=3, space="PSUM") as psot,
                tc.tile_pool(name="prbc", bufs=1, space="PSUM") as prbc,
            ):
                for h in range(NH):
                    for j in reversed(range(NTCH)):
                        ts = slice(j * TCH, (j + 1) * TCH)
                        n_i = 4 * j + 4
                        psum_l = psl.tile([1, TCH], F32, tag="l")
                        psum_ot = psot.tile([128, TCH], F32, tag="ot")
                        for i in range(n_i):
                            psum_st = pst.tile([128, TCH], F32, tag="st")
                            nc.tensor.matmul(
                                psum_st[:],
                                kt_sb[:, i * 128 : (i + 1) * 128],
                                qt_sb[h][:, ts],
                                start=True,
                                stop=True,
                            )
                            phat = phatp.tile([128, TCH], F32R, tag="phat")
                            nc.scalar.activation(
                                out=phat[:],
                                in_=psum_st[:],
                                func=mybir.ActivationFunctionType.Exp,
                                scale=SCALE,
                            )
                            r = i - 4 * j
                            if r >= 0:  # diagonal-crossing tile: zero where s > t
                                nc.gpsimd.affine_select(
                                    out=phat[:],
                                    in_=phat[:],
                                    compare_op=mybir.AluOpType.is_ge,
                                    fill=0.0,
                                    base=-128 * r,
                                    pattern=[[1, TCH]],
                                    channel_multiplier=-1,
                                )
                            nc.tensor.matmul(
                                psum_l[:],
                                ones_sb[:],
                                phat[:],
                                start=(i == 0),
                                stop=(i == n_i - 1),
                            )
                            nc.tensor.matmul(
                                psum_ot[:],
                                v_sb[:, i, :],
                                phat[:],
                                start=(i == 0),
                                stop=(i == n_i - 1),
                            )
                        recip_sb = recipp.tile([1, TCH], F32, tag="recip")
                        nc.vector.reciprocal_approx_fast(recip_sb[:], psum_l[:])
                        recip_bf = recipbp.tile([1, TCH], BF16, tag="recipb")
                        nc.vector.tensor_copy(recip_bf[:], recip_sb[:])
                        # partition-broadcast recip via K=1 bf16 matmul
                        prb = prbc.tile([128, TCH], F32, tag="prb")
                        nc.tensor.matmul(
                            prb[:],
                            onescol[:],
                            recip_bf[:],
                            start=True,
                            stop=True,
                        )
                        rbc = rbcp.tile([128, TCH], F32, tag="rbc")
                        nc.scalar.copy(rbc[:], prb[:])
                        nc.vector.tensor_mul(ao_sb[h][:, ts], psum_ot[:], rbc[:])

            # ---------------- phase 3: output projection ----------------
            with (
                tc.tile_pool(name="psy", bufs=1, space="PSUM") as psy,
                tc.tile_pool(name="ys", bufs=8) as ys,
            ):
                for tt in range(NST):
                    tsl = slice(tt * 128, (tt + 1) * 128)
                    pys = [
                        psy.tile([128, 512], F32, tag=f"y{fc}", name=f"y{fc}")
                        for fc in range(8)
                    ]
                    for hb in range(NH):
                        for fc in range(8):
                            fsl = slice(fc * 512, (fc + 1) * 512)
                            nc.tensor.matmul(
                                pys[fc][:],
                                ao_sb[hb][:, tsl],
                                wo_sb[:, hb, fsl],
                                start=(hb == 0),
                                stop=(hb == NH - 1),
                            )
                            if hb == NH - 1:
                                yt = ys.tile([128, 512], BF16, tag="yt")
                                if fc % 2 == 0:
                                    nc.vector.tensor_copy(yt[:], pys[fc][:])
                                else:
                                    nc.scalar.copy(yt[:], pys[fc][:])
                                nc.sync.dma_start(out=y[tsl, fsl], in_=yt[:])


def kernel(x, wq, wk, wv, wo):
    x = np.asarray(x, dtype=np.float32)
    wq = np.asarray(wq, dtype=np.float32)
    wk = np.asarray(wk, dtype=np.float32)
    wv = np.asarray(wv, dtype=np.float32)
    wo = np.asarray(wo, dtype=np.float32)

    if "nc" not in _CACHE:
        _CACHE["nc"] = _build()
    nc = _CACHE["nc"]

    xT = np.ascontiguousarray(x[0].T)  # [DIM, T]
    ones = np.ones((128, 1), np.float32)
    onescol = np.ones((1, 128), ml_dtypes.bfloat16)
    in_maps = []
    for c in range(NCORE):
        qs = slice(c * NH * HD, (c + 1) * NH * HD)
        ks = slice(c * HD, (c + 1) * HD)
        in_maps.append(
            {
                "xT": xT,
                "wqT": np.ascontiguousarray(wq[qs, :].T),
                "wkT": np.ascontiguousarray(wk[ks, :].T),
                "wvT": np.ascontiguousarray(wv[ks, :].T),
                "woT": np.ascontiguousarray(wo[:, qs].T),
                "ones": ones,
                "onescol": onescol,
            }
        )

    res = run_bass_kernel_spmd(
        nc, in_maps, core_ids=list(range(NCORE)), trace=TRACE
    )
    LAST["results"] = res

    out = np.zeros((T, DIM), dtype=np.float64)
    for c in range(NCORE):
        out += res.results[c]["y"].astype(np.float64)
    return out.astype(np.float32).reshape(1, T, DIM)


# revision 21
# speedup vs baseline: 1.0748x; 1.0748x over previous
"""GQA prefill attention (B=1, T=2048, DIM=4096, 32 q-heads / 8 kv-heads),
tensor-parallel over 8 NeuronCores.

Sharding: core c owns kv head c and its 4 query heads: wq rows
[512c, 512c+512), wk/wv rows [128c, 128c+128), wo cols [512c, 512c+512).
Each core computes a partial y = attn_c @ wo_c.T in [T, DIM]; the host sums
the 8 partials (the "all-reduce after wo").

All matmuls run in float32r (TF32-like; measured ~10% faster per row than
bf16 on this part). Everything on chip is kept transposed ([feature, t]
layouts) so that the PE contraction dim (partitions) always lines up without
any on-chip data transposes except V (16 PE-transposes of 128x128).

Per-core pipeline:
  phase 1: Q^T/K^T/V^T projections from x^T (host-pretransposed inputs);
           wo prefetched into SBUF during chunks 1-3
  phase 2: causal flash attention per head, s-on-partitions layout:
           S^T tiles -> exp (max-free softmax, scores are bounded) ->
           rowsum via ones-stationary matmul -> O^T accumulation ->
           reciprocal_approx_fast -> partition-broadcast via K=1 bf16
           matmul (no DRAM bounce) -> normalize into ao
  phase 3: y_partial = AO^T.T @ wo^T in [t, f] tiles, hb-outer psum
           accumulation over 8 banks, bf16 partials DMAd to DRAM
"""

import sys

sys.path.insert(0, "/opt/trn_rl_repo")

import ml_dtypes
import numpy as np

import concourse.bass as bass
import concourse.tile as tile
from concourse import bacc, mybir
from concourse.bass_utils import run_bass_kernel_spmd
from concourse.masks import make_identity

T = 2048
DIM = 4096
HD = 128
NCORE = 8
NH = 4  # q heads per core
TCH = 512
NTCH = T // TCH  # 4 t-chunks
NST = T // 128  # 16 s-tiles
NDT = DIM // 128  # 32 d-tiles
F32 = mybir.dt.float32
F32R = mybir.dt.float32r
BF16 = mybir.dt.bfloat16
SCALE = 1.0 / float(np.sqrt(HD))
NEG = -1e30

# test.py can flip these before calling kernel() to get profiling info
TRACE = False
LAST = {}

_CACHE = {}


def _build():
    nc = bacc.Bacc("TRN2", target_bir_lowering=False, debug=False, num_devices=NCORE)
    xT = nc.dram_tensor("xT", [DIM, T], BF16, kind="ExternalInput").ap()
    wqT = nc.dram_tensor("wqT", [DIM, NH * HD], F32, kind="ExternalInput").ap()
    wkT = nc.dram_tensor("wkT", [DIM, HD], F32, kind="ExternalInput").ap()
    wvT = nc.dram_tensor("wvT", [DIM, HD], F32, kind="ExternalInput").ap()
    woT = nc.dram_tensor("woT", [NH * HD, DIM], F32, kind="ExternalInput").ap()
    ones_in = nc.dram_tensor("ones", [128, 1], F32, kind="ExternalInput").ap()
    onescol_in = nc.dram_tensor("onescol", [1, 128], BF16, kind="ExternalInput").ap()
    y = nc.dram_tensor("y", [T, DIM], BF16, kind="ExternalOutput").ap()

    with tile.TileContext(nc) as tc:
        with tc.tile_pool(name="persist", bufs=1) as persist:
            qt_sb = [persist.tile([128, T], F32R, tag=f"qt{h}", name=f"qt{h}") for h in range(NH)]
            kt_sb = persist.tile([128, T], F32R, tag="kt")
            vt_sb = persist.tile([128, T], F32, tag="vt")
            ao_sb = [persist.tile([128, T], F32R, tag=f"ao{h}", name=f"ao{h}") for h in range(NH)]
            ones_sb = persist.tile([128, 1], F32R, tag="ones")
            onescol = persist.tile([1, 128], BF16, tag="onescol")
            ident = persist.tile([128, 128], F32, tag="ident")
            v_sb = persist.tile([128, NST, HD], F32R, tag="v")
            nc.sync.dma_start(out=ones_sb, in_=ones_in.bitcast(F32R))
            nc.sync.dma_start(out=onescol, in_=onescol_in)
            make_identity(nc, ident)
            wor = woT.rearrange("(hb p) f -> p hb f", p=128)

            # ---------------- phase 1: Q/K/V projections ----------------
            with (
                tc.tile_pool(name="w1", bufs=1) as w1,
                tc.tile_pool(name="xs", bufs=6) as xs,
                tc.tile_pool(name="xbfs", bufs=6) as xbfs,
                tc.tile_pool(name="psp", bufs=1, space="PSUM") as psp,
                tc.tile_pool(name="ptr1", bufs=1, space="PSUM") as ptr1,
            ):
                wq_sb = w1.tile([128, NDT, NH * HD], F32R, tag="wq")
                wk_sb = w1.tile([128, NDT, HD], F32R, tag="wk")
                wv_sb = w1.tile([128, NDT, HD], F32R, tag="wv")
                wqr = wqT.rearrange("(db p) f -> p db f", p=128)
                wkr = wkT.rearrange("(db p) f -> p db f", p=128)
                wvr = wvT.rearrange("(db p) f -> p db f", p=128)
                for ch in range(NTCH):
                    cs = slice(ch * TCH, (ch + 1) * TCH)
                    qps = [
                        psp.tile([128, TCH], F32, tag=f"projq{fq}", name=f"projq{fq}")
                        for fq in range(NH)
                    ]
                    kps = psp.tile([128, TCH], F32, tag="projk")
                    vps = psp.tile([128, TCH], F32, tag="projv")
                    for d in range(NDT):
                        if ch == 0 and (d in (0, 1) or (d >= 4 and d % 4 == 0)):
                            # first two tiny groups so the PE starts almost
                            # immediately; 1MB groups once streaming
                            g = slice(d, d + (1 if d == 0 else 3 if d == 1 else 4))
                            nc.scalar.dma_start(
                                out=wq_sb[:, g, :], in_=wqr[:, g, :].bitcast(F32R)
                            )
                            nc.scalar.dma_start(
                                out=wk_sb[:, g, :], in_=wkr[:, g, :].bitcast(F32R)
                            )
                            nc.scalar.dma_start(
                                out=wv_sb[:, g, :], in_=wvr[:, g, :].bitcast(F32R)
                            )
                        xbf = xbfs.tile([128, TCH], BF16, tag="xbf")
                        nc.sync.dma_start(
                            out=xbf, in_=xT[d * 128 : (d + 1) * 128, cs]
                        )
                        xt = xs.tile([128, TCH], F32R, tag="xt")
                        if d % 2 == 0:
                            nc.vector.tensor_copy(xt[:], xbf[:])
                        else:
                            nc.scalar.copy(xt[:], xbf[:])
                        st = d == 0
                        sp = d == NDT - 1
                        for fq in range(NH):
                            nc.tensor.matmul(
                                qps[fq][:],
                                wq_sb[:, d, fq * HD : (fq + 1) * HD],
                                xt[:],
                                start=st,
                                stop=sp,
                            )
                        nc.tensor.matmul(
                            kps[:], wk_sb[:, d, :], xt[:], start=st, stop=sp
                        )
                        nc.tensor.matmul(
                            vps[:], wv_sb[:, d, :], xt[:], start=st, stop=sp
                        )
                    for fq in range(NH):
                        nc.scalar.copy(qt_sb[fq][:, cs], qps[fq][:])
                    nc.vector.tensor_copy(kt_sb[:, cs], kps[:])
                    nc.vector.tensor_copy(vt_sb[:, cs], vps[:])
                    for ii in range(4 * ch, 4 * ch + 4):
                        ptr = ptr1.tile([128, HD], F32, tag="tr")
                        nc.tensor.transpose(
                            ptr[:], vt_sb[:, ii * 128 : (ii + 1) * 128], ident[:]
                        )
                        nc.vector.tensor_copy(v_sb[:, ii, :], ptr[:])

            # ---------------- phase 2 + 3 weights ----------------
            with tc.tile_pool(name="w2", bufs=1) as w2:
                wo_sb = w2.tile([128, NH, DIM], F32R, tag="wo")
                for hb in range(NH):
                    nc.scalar.dma_start(
                        out=wo_sb[:, hb, :], in_=wor[:, hb, :].bitcast(F32R)
                    )
                _phases23(nc, tc, qt_sb, kt_sb, v_sb, ao_sb, ones_sb, onescol,
                          wo_sb, y)

    nc.compile()
    return nc


def _phases23(nc, tc, qt_sb, kt_sb, v_sb, ao_sb, ones_sb, onescol, wo_sb, y):
    if True:
            # ---------------- phase 2: attention ----------------
            with (
                tc.tile_pool(name="phatp", bufs=6) as phatp,
                tc.tile_pool(name="recipp", bufs=2) as recipp,
                tc.tile_pool(name="recipbp", bufs=2) as recipbp,
                tc.tile_pool(name="rbcp", bufs=2) as rbcp,
                tc.tile_pool(name="pst", bufs=3, space="PSUM") as pst,
                tc.tile_pool(name="psl", bufs=1, space="PSUM") as psl,
                tc.tile_pool(name="psot", bufs=3, space="PSUM") as psot,
                tc.tile_pool(name="prbc", bufs=1, space="PSUM") as prbc,
            ):
                for h in range(NH):
                    for j in reversed(range(NTCH)):
                        ts = slice(j * TCH, (j + 1) * TCH)
                        n_i = 4 * j + 4
                        psum_l = psl.tile([1, TCH], F32, tag="l")
                        psum_ot = psot.tile([128, TCH], F32, tag="ot")
                        for i in range(n_i):
                            psum_st = pst.tile([128, TCH], F32, tag="st")
                            nc.tensor.matmul(
                                psum_st[:],
                                kt_sb[:, i * 128 : (i + 1) * 128],
                                qt_sb[h][:, ts],
                                start=True,
                                stop=True,
                            )
                            phat = phatp.tile([128, TCH], F32R, tag="phat")
                            nc.scalar.activation(
                                out=phat[:],
                                in_=psum_st[:],
                                func=mybir.ActivationFunctionType.Exp,
                                scale=SCALE,
                            )
                            r = i - 4 * j
                            if r >= 0:  # diagonal-crossing tile: zero where s > t
                                nc.gpsimd.affine_select(
                                    out=phat[:],
                                    in_=phat[:],
                                    compare_op=mybir.AluOpType.is_ge,
                                    fill=0.0,
                                    base=-128 * r,
                                    pattern=[[1, TCH]],
                                    channel_multiplier=-1,
                                )
                            nc.tensor.matmul(
                                psum_l[:],
                                ones_sb[:],
                                phat[:],
                                start=(i == 0),
                                stop=(i == n_i - 1),
                            )
                            nc.tensor.matmul(
                                psum_ot[:],
                                v_sb[:, i, :],
                                phat[:],
                                start=(i == 0),
                                stop=(i == n_i - 1),
                            )
                        recip_sb = recipp.tile([1, TCH], F32, tag="recip")
                        nc.vector.reciprocal_approx_fast(recip_sb[:], psum_l[:])
                        recip_bf = recipbp.tile([1, TCH], BF16, tag="recipb")
                        nc.vector.tensor_copy(recip_bf[:], recip_sb[:])
                        # partition-broadcast recip via K=1 bf16 matmul
                        prb = prbc.tile([128, TCH], F32, tag="prb")
                        nc.tensor.matmul(
                            prb[:],
                            onescol[:],
                            recip_bf[:],
                            start=True,
                            stop=True,
                        )
                        rbc = rbcp.tile([128, TCH], F32, tag="rbc")
                        nc.scalar.copy(rbc[:], prb[:])
                        nc.vector.tensor_mul(ao_sb[h][:, ts], psum_ot[:], rbc[:])

            # ---------------- phase 3: output projection ----------------
            with (
                tc.tile_pool(name="psy", bufs=1, space="PSUM") as psy,
                tc.tile_pool(name="ys", bufs=8) as ys,
            ):
                for tt in range(NST):
                    tsl = slice(tt * 128, (tt + 1) * 128)
                    pys = [
                        psy.tile([128, 512], F32, tag=f"y{fc}", name=f"y{fc}")
                        for fc in range(8)
                    ]
                    for hb in range(NH):
                        for fc in range(8):
                            fsl = slice(fc * 512, (fc + 1) * 512)
                            nc.tensor.matmul(
                                pys[fc][:],
                                ao_sb[hb][:, tsl],
                                wo_sb[:, hb, fsl],
                                start=(hb == 0),
                                stop=(hb == NH - 1),
                            )
                            if hb == NH - 1:
                                yt = ys.tile([128, 512], BF16, tag="yt")
                                if fc % 2 == 0:
                                    nc.vector.tensor_copy(yt[:], pys[fc][:])
                                else:
                                    nc.scalar.copy(yt[:], pys[fc][:])
                                nc.sync.dma_start(out=y[tsl, fsl], in_=yt[:])


def kernel(x, wq, wk, wv, wo):
    x = np.asarray(x, dtype=np.float32)
    wq = np.asarray(wq, dtype=np.float32)
    wk = np.asarray(wk, dtype=np.float32)
    wv = np.asarray(wv, dtype=np.float32)
    wo = np.asarray(wo, dtype=np.float32)

    if "nc" not in _CACHE:
        _CACHE["nc"] = _build()
    nc = _CACHE["nc"]

    xT = np.ascontiguousarray(x[0].T).astype(ml_dtypes.bfloat16)  # [DIM, T]
    ones = np.ones((128, 1), np.float32)
    onescol = np.ones((1, 128), ml_dtypes.bfloat16)
    in_maps = []
    for c in range(NCORE):
        qs = slice(c * NH * HD, (c + 1) * NH * HD)
        ks = slice(c * HD, (c + 1) * HD)
        in_maps.append(
            {
                "xT": xT,
                "wqT": np.ascontiguousarray(wq[qs, :].T),
                "wkT": np.ascontiguousarray(wk[ks, :].T),
                "wvT": np.ascontiguousarray(wv[ks, :].T),
                "woT": np.ascontiguousarray(wo[:, qs].T),
                "ones": ones,
                "onescol": onescol,
            }
        )

    res = run_bass_kernel_spmd(
        nc, in_maps, core_ids=list(range(NCORE)), trace=TRACE
    )
    LAST["results"] = res

    out = np.zeros((T, DIM), dtype=np.float64)
    for c in range(NCORE):
        out += res.results[c]["y"].astype(np.float64)
    return out.astype(np.float32).reshape(1, T, DIM)


# revision 22
# speedup vs baseline: 1.0819x; 1.0066x over previous
"""GQA prefill attention (B=1, T=2048, DIM=4096, 32 q-heads / 8 kv-heads),
tensor-parallel over 8 NeuronCores.

Sharding: core c owns kv head c and its 4 query heads: wq rows
[512c, 512c+512), wk/wv rows [128c, 128c+128), wo cols [512c, 512c+512).
Each core computes a partial y = attn_c @ wo_c.T in [T, DIM]; the host sums
the 8 partials (the "all-reduce after wo").

All matmuls run in float32r (TF32-like; measured ~10% faster per row than
bf16 on this part). Everything on chip is kept transposed ([feature, t]
layouts) so that the PE contraction dim (partitions) always lines up without
any on-chip data transposes except V (16 PE-transposes of 128x128).

Per-core pipeline:
  phase 1: Q^T/K^T/V^T projections from x^T (host-pretransposed inputs);
           wo prefetched into SBUF during chunks 1-3
  phase 2: causal flash attention per head, s-on-partitions layout:
           S^T tiles -> exp (max-free softmax, scores are bounded) ->
           rowsum via ones-stationary matmul -> O^T accumulation ->
           reciprocal_approx_fast -> partition-broadcast via K=1 bf16
           matmul (no DRAM bounce) -> normalize into ao
  phase 3: y_partial = AO^T.T @ wo^T in [t, f] tiles, hb-outer psum
           accumulation over 8 banks, bf16 partials DMAd to DRAM
"""

import sys

sys.path.insert(0, "/opt/trn_rl_repo")

import ml_dtypes
import numpy as np

import concourse.bass as bass
import concourse.tile as tile
from concourse import bacc, mybir
from concourse.bass_utils import run_bass_kernel_spmd
from concourse.masks import make_identity

T = 2048
DIM = 4096
HD = 128
NCORE = 8
NH = 4  # q heads per core
TCH = 512
NTCH = T // TCH  # 4 t-chunks
NST = T // 128  # 16 s-tiles
NDT = DIM // 128  # 32 d-tiles
F32 = mybir.dt.float32
F32R = mybir.dt.float32r
BF16 = mybir.dt.bfloat16
SCALE = 1.0 / float(np.sqrt(HD))
NEG = -1e30

# test.py can flip these before calling kernel() to get profiling info
TRACE = False
LAST = {}

_CACHE = {}


def _build():
    nc = bacc.Bacc("TRN2", target_bir_lowering=False, debug=False, num_devices=NCORE)
    xT = nc.dram_tensor("xT", [DIM, T], BF16, kind="ExternalInput").ap()
    wqT = nc.dram_tensor("wqT", [DIM, NH * HD], F32, kind="ExternalInput").ap()
    wkT = nc.dram_tensor("wkT", [DIM, HD], F32, kind="ExternalInput").ap()
    wvT = nc.dram_tensor("wvT", [DIM, HD], F32, kind="ExternalInput").ap()
    woT = nc.dram_tensor("woT", [NH * HD, DIM], F32, kind="ExternalInput").ap()
    ones_in = nc.dram_tensor("ones", [128, 1], F32, kind="ExternalInput").ap()
    onescol_in = nc.dram_tensor("onescol", [1, 128], BF16, kind="ExternalInput").ap()
    y = nc.dram_tensor("y", [T, DIM], BF16, kind="ExternalOutput").ap()

    with tile.TileContext(nc) as tc:
        with tc.tile_pool(name="persist", bufs=1) as persist:
            qt_sb = [persist.tile([128, T], F32R, tag=f"qt{h}", name=f"qt{h}") for h in range(NH)]
            kt_sb = persist.tile([128, T], F32R, tag="kt")
            vt_sb = persist.tile([128, T], F32, tag="vt")
            ao_sb = [persist.tile([128, T], F32R, tag=f"ao{h}", name=f"ao{h}") for h in range(NH)]
            ones_sb = persist.tile([128, 1], F32R, tag="ones")
            onescol = persist.tile([1, 128], BF16, tag="onescol")
            ident = persist.tile([128, 128], F32, tag="ident")
            v_sb = persist.tile([128, NST, HD], F32R, tag="v")
            nc.sync.dma_start(out=ones_sb, in_=ones_in.bitcast(F32R))
            nc.sync.dma_start(out=onescol, in_=onescol_in)
            make_identity(nc, ident)
            expwarm = persist.tile([1, 2], F32, tag="expwarm")
            nc.vector.memset(expwarm, 0.0)
            nc.scalar.activation(
                out=expwarm[:],
                in_=expwarm[:],
                func=mybir.ActivationFunctionType.Exp,
                scale=1.0,
            )
            wor = woT.rearrange("(hb p) f -> p hb f", p=128)

            # ---------------- phase 1: Q/K/V projections ----------------
            with (
                tc.tile_pool(name="w1", bufs=1) as w1,
                tc.tile_pool(name="xs", bufs=6) as xs,
                tc.tile_pool(name="xbfs", bufs=6) as xbfs,
                tc.tile_pool(name="psp", bufs=1, space="PSUM") as psp,
                tc.tile_pool(name="ptr1", bufs=1, space="PSUM") as ptr1,
            ):
                wq_sb = w1.tile([128, NDT, NH * HD], F32R, tag="wq")
                wk_sb = w1.tile([128, NDT, HD], F32R, tag="wk")
                wv_sb = w1.tile([128, NDT, HD], F32R, tag="wv")
                wqr = wqT.rearrange("(db p) f -> p db f", p=128)
                wkr = wkT.rearrange("(db p) f -> p db f", p=128)
                wvr = wvT.rearrange("(db p) f -> p db f", p=128)
                for ch in range(NTCH):
                    cs = slice(ch * TCH, (ch + 1) * TCH)
                    qps = [
                        psp.tile([128, TCH], F32, tag=f"projq{fq}", name=f"projq{fq}")
                        for fq in range(NH)
                    ]
                    kps = psp.tile([128, TCH], F32, tag="projk")
                    vps = psp.tile([128, TCH], F32, tag="projv")
                    for d in range(NDT):
                        if ch == 0 and (d in (0, 1) or (d >= 4 and d % 4 == 0)):
                            # first two tiny groups so the PE starts almost
                            # immediately; 1MB groups once streaming
                            g = slice(d, d + (1 if d == 0 else 3 if d == 1 else 4))
                            nc.scalar.dma_start(
                                out=wq_sb[:, g, :], in_=wqr[:, g, :].bitcast(F32R)
                            )
                            nc.scalar.dma_start(
                                out=wk_sb[:, g, :], in_=wkr[:, g, :].bitcast(F32R)
                            )
                            nc.scalar.dma_start(
                                out=wv_sb[:, g, :], in_=wvr[:, g, :].bitcast(F32R)
                            )
                        xbf = xbfs.tile([128, TCH], BF16, tag="xbf")
                        nc.sync.dma_start(
                            out=xbf, in_=xT[d * 128 : (d + 1) * 128, cs]
                        )
                        xt = xs.tile([128, TCH], F32R, tag="xt")
                        if d % 2 == 0:
                            nc.vector.tensor_copy(xt[:], xbf[:])
                        else:
                            nc.scalar.copy(xt[:], xbf[:])
                        st = d == 0
                        sp = d == NDT - 1
                        for fq in range(NH):
                            nc.tensor.matmul(
                                qps[fq][:],
                                wq_sb[:, d, fq * HD : (fq + 1) * HD],
                                xt[:],
                                start=st,
                                stop=sp,
                            )
                        nc.tensor.matmul(
                            kps[:], wk_sb[:, d, :], xt[:], start=st, stop=sp
                        )
                        nc.tensor.matmul(
                            vps[:], wv_sb[:, d, :], xt[:], start=st, stop=sp
                        )
                    for fq in range(NH):
                        nc.scalar.copy(qt_sb[fq][:, cs], qps[fq][:])
                    nc.vector.tensor_copy(kt_sb[:, cs], kps[:])
                    nc.vector.tensor_copy(vt_sb[:, cs], vps[:])
                    for ii in range(4 * ch, 4 * ch + 4):
                        ptr = ptr1.tile([128, HD], F32, tag="tr")
                        nc.tensor.transpose(
                            ptr[:], vt_sb[:, ii * 128 : (ii + 1) * 128], ident[:]
                        )
                        nc.vector.tensor_copy(v_sb[:, ii, :], ptr[:])

            # ---------------- phase 2 + 3 weights ----------------
            with tc.tile_pool(name="w2", bufs=1) as w2:
                wo_sb = w2.tile([128, NH, DIM], F32R, tag="wo")
                for hb in range(NH):
                    nc.scalar.dma_start(
                        out=wo_sb[:, hb, :], in_=wor[:, hb, :].bitcast(F32R)
                    )
                _phases23(nc, tc, qt_sb, kt_sb, v_sb, ao_sb, ones_sb, onescol,
                          wo_sb, y)

    nc.compile()
    return nc


def _phases23(nc, tc, qt_sb, kt_sb, v_sb, ao_sb, ones_sb, onescol, wo_sb, y):
    if True:
            # ---------------- phase 2: attention ----------------
            with (
                tc.tile_pool(name="phatp", bufs=6) as phatp,
                tc.tile_pool(name="recipp", bufs=2) as recipp,
                tc.tile_pool(name="recipbp", bufs=2) as recipbp,
                tc.tile_pool(name="rbcp", bufs=2) as rbcp,
                tc.tile_pool(name="pst", bufs=3, space="PSUM") as pst,
                tc.tile_pool(name="psl", bufs=1, space="PSUM") as psl,
                tc.tile_pool(name="psot", bufs=3, space="PSUM") as psot,
                tc.tile_pool(name="prbc", bufs=1, space="PSUM") as prbc,
            ):
                for h in range(NH):
                    for j in range(NTCH):
                        ts = slice(j * TCH, (j + 1) * TCH)
                        n_i = 4 * j + 4
                        psum_l = psl.tile([1, TCH], F32, tag="l")
                        psum_ot = psot.tile([128, TCH], F32, tag="ot")
                        for i in range(n_i):
                            psum_st = pst.tile([128, TCH], F32, tag="st")
                            nc.tensor.matmul(
                                psum_st[:],
                                kt_sb[:, i * 128 : (i + 1) * 128],
                                qt_sb[h][:, ts],
                                start=True,
                                stop=True,
                            )
                            phat = phatp.tile([128, TCH], F32R, tag="phat")
                            nc.scalar.activation(
                                out=phat[:],
                                in_=psum_st[:],
                                func=mybir.ActivationFunctionType.Exp,
                                scale=SCALE,
                            )
                            r = i - 4 * j
                            if r >= 0:  # diagonal-crossing tile: zero where s > t
                                nc.gpsimd.affine_select(
                                    out=phat[:],
                                    in_=phat[:],
                                    compare_op=mybir.AluOpType.is_ge,
                                    fill=0.0,
                                    base=-128 * r,
                                    pattern=[[1, TCH]],
                                    channel_multiplier=-1,
                                )
                            nc.tensor.matmul(
                                psum_l[:],
                                ones_sb[:],
                                phat[:],
                                start=(i == 0),
                                stop=(i == n_i - 1),
                            )
                            nc.tensor.matmul(
                                psum_ot[:],
                                v_sb[:, i, :],
                                phat[:],
                                start=(i == 0),
                                stop=(i == n_i - 1),
                            )
                        recip_sb = recipp.tile([1, TCH], F32, tag="recip")
                        nc.vector.reciprocal_approx_fast(recip_sb[:], psum_l[:])
                        recip_bf = recipbp.tile([1, TCH], BF16, tag="recipb")
                        nc.vector.tensor_copy(recip_bf[:], recip_sb[:])
                        # partition-broadcast recip via K=1 bf16 matmul
                        prb = prbc.tile([128, TCH], F32, tag="prb")
                        nc.tensor.matmul(
                            prb[:],
                            onescol[:],
                            recip_bf[:],
                            start=True,
                            stop=True,
                        )
                        rbc = rbcp.tile([128, TCH], F32, tag="rbc")
                        nc.scalar.copy(rbc[:], prb[:])
                        nc.vector.tensor_mul(ao_sb[h][:, ts], psum_ot[:], rbc[:])

            # ---------------- phase 3: output projection ----------------
            with (
                tc.tile_pool(name="psy", bufs=1, space="PSUM") as psy,
                tc.tile_pool(name="ys", bufs=8) as ys,
            ):
                for tt in range(NST):
                    tsl = slice(tt * 128, (tt + 1) * 128)
                    pys = [
                        psy.tile([128, 512], F32, tag=f"y{fc}", name=f"y{fc}")
                        for fc in range(8)
                    ]
                    for hb in range(NH):
                        for fc in range(8):
                            fsl = slice(fc * 512, (fc + 1) * 512)
                            nc.tensor.matmul(
                                pys[fc][:],
                                ao_sb[hb][:, tsl],
                                wo_sb[:, hb, fsl],
                                start=(hb == 0),
                                stop=(hb == NH - 1),
                            )
                            if hb == NH - 1:
                                yt = ys.tile([128, 512], BF16, tag="yt")
                                if fc % 2 == 0:
                                    nc.vector.tensor_copy(yt[:], pys[fc][:])
                                    nc.sync.dma_start(out=y[tsl, fsl], in_=yt[:])
                                else:
                                    nc.scalar.copy(yt[:], pys[fc][:])
                                    nc.scalar.dma_start(out=y[tsl, fsl], in_=yt[:])


def kernel(x, wq, wk, wv, wo):
    x = np.asarray(x, dtype=np.float32)
    wq = np.asarray(wq, dtype=np.float32)
    wk = np.asarray(wk, dtype=np.float32)
    wv = np.asarray(wv, dtype=np.float32)
    wo = np.asarray(wo, dtype=np.float32)

    if "nc" not in _CACHE:
        _CACHE["nc"] = _build()
    nc = _CACHE["nc"]

    xT = np.ascontiguousarray(x[0].T).astype(ml_dtypes.bfloat16)  # [DIM, T]
    ones = np.ones((128, 1), np.float32)
    onescol = np.ones((1, 128), ml_dtypes.bfloat16)
    in_maps = []
    for c in range(NCORE):
        qs = slice(c * NH * HD, (c + 1) * NH * HD)
        ks = slice(c * HD, (c + 1) * HD)
        in_maps.append(
            {
                "xT": xT,
                "wqT": np.ascontiguousarray(wq[qs, :].T),
                "wkT": np.ascontiguousarray(wk[ks, :].T),
                "wvT": np.ascontiguousarray(wv[ks, :].T),
                "woT": np.ascontiguousarray(wo[:, qs].T),
                "ones": ones,
                "onescol": onescol,
            }
        )

    res = run_bass_kernel_spmd(
        nc, in_maps, core_ids=list(range(NCORE)), trace=TRACE
    )
    LAST["results"] = res

    out = np.zeros((T, DIM), dtype=np.float64)
    for c in range(NCORE):
        out += res.results[c]["y"].astype(np.float64)
    return out.astype(np.float32).reshape(1, T, DIM)
